# revision 123
# baseline (speedup 1.0000x reference)
"""Trainium2 Bass kernel for nn_MixtureOfAttentionHeads.

Sharding: 8 cores = 4 batches x 2 heads. Core c handles batch c//2, head c%2:
it computes all three attention types (global/rel/local) for its head over the
full sequence, applies the router gate per token, then pairwise ReduceScatters
sum the two heads (0.5 factor folded into Wv); each core projects its owned
token blocks with W_o and writes them to DRAM.

Main optimizations over the straightforward version:
- Q/K projections in fp8e4m3 DoubleRow mode (4 accumulation steps over the
  256-deep double rows instead of 8, at 0.5 cycles/row). Weights are
  pre-scaled x32 on the host; the 32*32=1024 score scale folds into the
  exp scale, so fp8 costs no extra on-chip ops.
- V/router projection stays f32r: top-k routing decisions flip on ~1e-3
  logit perturbations and single flips produce O(10%) point errors, so the
  router path needs near-fp32 logits.
- Rel-position bias and causal/local masks are applied MULTIPLICATIVELY
  after the exp (es *= exp(bias) tiles precomputed on host) instead of as
  additive mask matmuls - frees the PE dispatch stream.
- Interleaved block ownership (rank h owns blocks with block%2==h) so every
  ReduceScatter piece carries tokens of both ranks; 3 pieces sized (4,8,4)
  blocks fire as their chunks finish, hiding all but the last collective's
  15us fixed cost.
- Per-chunk schedule staged as S->exp (only needs Q/K) ahead of the V
  projection so the tensor engine never idles on the f32r x DMA stream, and
  late chunks interleave next-type S phases with AV phases to keep the
  activation engine's exp stream saturated.

The SPMD program is identical on every core; all per-core differences
(batch, head weights, rel-bias tiles) arrive as input data.
"""

import os
import sys

sys.path.insert(0, "/opt/trn_rl_repo")

import numpy as np
import ml_dtypes

# --- problem constants (hardcoded per contract) ---
B, T, D, DH = 4, 2048, 1024, 128
H, NT, TOPK = 2, 3, 2
WIN, MRP = 128, 32
SCALE = float(np.sqrt(DH))
NB = T // 128          # 16 query/key blocks
NCH = T // 512         # 4 query chunks of 512
WS = 32.0              # fp8 weight prescale; scores come out x(WS*WS)
K2 = WS * WS           # 1024 score scale

_CACHE = {}

# RS pieces: (first block, nblocks). Rank h owns blocks with block%2==h, so
# every piece carries tokens for both ranks of the pair.
PIECES = [(0, 4), (4, 8), (12, 4)]


def _build_nc():
    import concourse.bass as bass  # noqa: F401
    import concourse.mybir as mybir
    import concourse.tile as tile
    from concourse import bacc
    from concourse.masks import make_identity
    from contextlib import ExitStack

    dt = mybir.dt
    f32, f32r, bf16, f8 = dt.float32, dt.float32r, dt.bfloat16, dt.float8e4
    AX = mybir.AxisListType
    ALU = mybir.AluOpType
    ACTF = mybir.ActivationFunctionType
    DR = mybir.MatmulPerfMode.DoubleRow

    nc = bacc.Bacc("TRN2", target_bir_lowering=False, num_devices=8)

    xT = nc.dram_tensor("xT", [D, T], f32r, kind="ExternalInput")
    x8T = nc.dram_tensor("x8T", [D, T], f8, kind="ExternalInput")
    wq = nc.dram_tensor("wq", [NT, D, DH], f8, kind="ExternalInput")
    wk = nc.dram_tensor("wk", [NT, D, DH], f8, kind="ExternalInput")
    wv = nc.dram_tensor("wv", [D, NT * DH + NT + 1], f32r, kind="ExternalInput")
    masks = nc.dram_tensor("masks", [NT, 2, 128, 128], bf16, kind="ExternalInput")
    biasv = nc.dram_tensor("biasv", [128, NT], f32, kind="ExternalInput")
    bb = nc.dram_tensor("bb", [128, NT], f32, kind="ExternalInput")
    Wo = nc.dram_tensor("Wo", [DH, D], bf16, kind="ExternalInput")
    wob = nc.dram_tensor("wob", [128, D], f32, kind="ExternalInput")
    out = nc.dram_tensor("out", [T // 2, D], f32, kind="ExternalOutput")
    dbg = os.environ.get("KERNEL_DEBUG", "0") == "1"
    if dbg:
        dbg_wgt = nc.dram_tensor("dbg_wgt", [128, NB, NT], f32, kind="ExternalOutput")

    NV = NT * DH + NT + 1  # 388 (even, fp32r requirement)

    with tile.TileContext(nc) as tc, ExitStack() as ctx:
        persist = ctx.enter_context(tc.tile_pool(name="persist", bufs=1))
        xsp = ctx.enter_context(tc.tile_pool(name="xsp", bufs=12))
        x8p = ctx.enter_context(tc.tile_pool(name="x8p", bufs=3))
        gatep = ctx.enter_context(tc.tile_pool(name="gatep", bufs=8))
        expp = ctx.enter_context(tc.tile_pool(name="expp", bufs=44))
        avs = ctx.enter_context(tc.tile_pool(name="avs", bufs=6))
        dram = ctx.enter_context(tc.tile_pool(name="dram", bufs=1, space="DRAM"))
        php = ctx.enter_context(tc.tile_pool(name="php", bufs=2, space="PSUM"))
        phv = ctx.enter_context(tc.tile_pool(name="phv", bufs=2, space="PSUM"))
        sp = ctx.enter_context(tc.tile_pool(name="sp", bufs=2, space="PSUM"))
        avp = ctx.enter_context(tc.tile_pool(name="avp", bufs=2, space="PSUM"))

        # ---- persistent SBUF tensors ----
        qT = [persist.tile([128, T], f32r, tag=f"qT{t}", name=f"qT{t}") for t in range(NT)]
        kT = [persist.tile([128, T], f32r, tag=f"kT{t}", name=f"kT{t}") for t in range(NT)]
        V3 = persist.tile([128, NB, NT, DH + 2], bf16, tag="V3")
        wgt = persist.tile([128, NB, NT], f32, tag="wgt")
        comb = persist.tile([128, NB, DH], f32, tag="comb")
        comb16 = persist.tile([128, NB, DH], bf16, tag="comb16")
        # [dh, parity, j, tok]: block 2j+hh lives at [:, hh, j, :]
        combT = persist.tile([128, 2, NB // 2, 128], bf16, tag="combT")
        mask_sb = persist.tile([128, NT, 2, 128], bf16, tag="masks")
        biasv_sb = persist.tile([128, NT], f32, tag="biasv")
        bb_sb = persist.tile([128, NT], f32, tag="bb")
        Wo_sb = persist.tile([128, D], bf16, tag="Wo")
        wob_sb = persist.tile([128, D], f32, tag="wob")
        ident16 = persist.tile([128, 128], bf16, tag="ident16")

        wq_sb = persist.tile([128, NT, 8, DH], f8, tag="wq")
        wk_sb = persist.tile([128, NT, 8, DH], f8, tag="wk")
        wv_sb = persist.tile([128, 8, NV], f32r, tag="wv")

        def load_consts():
            # wv (big, needed early) on the scalar HWDGE queue; the other
            # constants ride the otherwise-idle SWDGE (gpsimd/Pool) path so
            # they don't contend with the x stream on the DMA engines
            nc.scalar.dma_start(
                out=wv_sb, in_=wv.rearrange("(k p) m -> p k m", p=128)
            )
            for t in range(NT):
                for kind in range(2):
                    nc.scalar.dma_start(
                        out=mask_sb[:, t, kind, :], in_=masks[t, kind, :, :]
                    )
            nc.scalar.dma_start(out=biasv_sb, in_=biasv[:, :])
            nc.scalar.dma_start(out=bb_sb, in_=bb[:, :])
            # Wo/wob are needed only by the output projection (~45us in);
            # ship them on the idle SWDGE path off the main DMA engines
            nc.gpsimd.dma_start(out=Wo_sb, in_=Wo[:, :])
            nc.gpsimd.dma_start(out=wob_sb, in_=wob[:, :])

        make_identity(nc, ident16)
        nc.vector.memset(V3[:, :, :, DH : DH + 2], 0.0)
        nc.vector.memset(V3[:, :, :, DH : DH + 1], 1.0)

        rs_in = [
            dram.tile([2 * 128, 64 * nb], bf16, name=f"rs_in{p}")
            for p, (_, nb) in enumerate(PIECES)
        ]
        rs_out = [
            dram.tile([128, 64 * nb], bf16, name=f"rs_out{p}")
            for p, (_, nb) in enumerate(PIECES)
        ]

        def load_qk_weights(t):
            nc.sync.dma_start(
                out=wq_sb[:, t, :, :],
                in_=wq[t].rearrange("(k p) m -> p k m", p=128),
            )
            nc.scalar.dma_start(
                out=wk_sb[:, t, :, :],
                in_=wk[t].rearrange("(k p) m -> p k m", p=128),
            )

        xs_tiles = {}

        x8_tiles = {}

        def load_slab(c4):
            """DMA the fp8 + f32r x tiles for token slab c4 (512 toks)."""
            sl = slice(512 * c4, 512 * (c4 + 1))
            x8t = x8p.tile([128, 8, 512], f8, tag="x8", name=f"x8_{c4}")
            for k in range(2):
                nc.sync.dma_start(
                    out=x8t[:, 4 * k : 4 * k + 4, :],
                    in_=x8T.rearrange("(k p) m -> p k m", p=128)[
                        :, 4 * k : 4 * k + 4, sl
                    ],
                )
            x8_tiles[c4] = x8t
            xs = []
            for k in range(8):
                xk = xsp.tile([128, 512], f32r, tag="xs", name=f"xs_{c4}_{k}")
                nc.sync.dma_start(
                    out=xk, in_=xT[128 * k : 128 * (k + 1), sl]
                )
                xs.append(xk)
            xs_tiles[c4] = xs

        def qk_part(c4):
            """fp8 DoubleRow Q/K projections for slab c4.

            NOTE: every weight-DMA must be ISSUED before the matmuls that
            read it (slab 0 consumes all three types) — first-run reads of
            not-yet-written SBUF otherwise.
            """
            sl = slice(512 * c4, 512 * (c4 + 1))
            x8t = x8_tiles.pop(c4)
            if c4 == 0:
                load_qk_weights(1)
                load_qk_weights(2)
            for t in range(NT):
                psq = php.tile([128, 512], f32, tag="php")
                for k in range(4):
                    nc.tensor.matmul(
                        psq,
                        wq_sb[:, t, 2 * k : 2 * k + 2, :],
                        x8t[:, 2 * k : 2 * k + 2, :],
                        start=(k == 0), stop=(k == 3),
                        perf_mode=DR,
                    )
                nc.scalar.copy(qT[t][:, sl], psq)
                psk = php.tile([128, 512], f32, tag="php")
                for k in range(4):
                    nc.tensor.matmul(
                        psk,
                        wk_sb[:, t, 2 * k : 2 * k + 2, :],
                        x8t[:, 2 * k : 2 * k + 2, :],
                        start=(k == 0), stop=(k == 3),
                        perf_mode=DR,
                    )
                nc.vector.tensor_copy(kT[t][:, sl], psk)

        def v_part(c4):
            """V + router projection and gates for slab c4 (needs xs)."""
            xs = xs_tiles.pop(c4)
            lg4 = gatep.tile([128, 4, NT], f32, tag="lg4")
            for ib in range(4):
                i = 4 * c4 + ib
                ibo = 128 * ib
                psv = phv.tile([128, NV], f32, tag="phv")
                for k in range(8):
                    nc.tensor.matmul(
                        psv,
                        xs[k][:, ibo : ibo + 128],
                        wv_sb[:, k, :],
                        start=(k == 0),
                        stop=(k == 7),
                    )
                nc.vector.tensor_copy(V3[:, i, :, 0:DH], psv[:, 0 : NT * DH])
                nc.vector.tensor_add(
                    lg4[:, ib, :], psv[:, NT * DH : NT * DH + NT], bb_sb
                )
            # vectorized gate pipeline over the whole slab (4 blocks at once)
            mn = gatep.tile([128, 4, 1], f32, tag="mn")
            nc.vector.tensor_tensor(
                mn, lg4[:, :, 0:1], lg4[:, :, 1:2], op=ALU.min
            )
            mn2 = gatep.tile([128, 4, 1], f32, tag="mn2")
            nc.vector.tensor_tensor(mn2, mn, lg4[:, :, 2:3], op=ALU.min)
            eg = gatep.tile([128, 4, NT], f32, tag="eg")
            nc.scalar.activation(eg, lg4, ACTF.Exp)
            gt = gatep.tile([128, 4, NT], f32, tag="gt")
            nc.vector.tensor_tensor(
                gt, lg4, mn2.broadcast_to([128, 4, NT]), op=ALU.is_gt
            )
            ew = gatep.tile([128, 4, NT], f32, tag="ew")
            nc.vector.tensor_mul(ew, eg, gt)
            sm = gatep.tile([128, 4, 1], f32, tag="sm")
            nc.vector.tensor_reduce(sm, ew, axis=AX.X, op=ALU.add)
            rc = gatep.tile([128, 4, 1], f32, tag="rc")
            nc.vector.reciprocal(rc, sm)
            nc.vector.tensor_mul(
                wgt[:, 4 * c4 : 4 * c4 + 4, :], ew, rc.broadcast_to([128, 4, NT])
            )

        def s_phase(t, c4):
            """S^T -> exp -> masked es tiles for type t, query chunk c4."""
            is_causal = t < 2
            has_prev = t >= 1
            base = 4 * c4
            jlo_c = 0 if is_causal else max(0, base - 1)
            es_tiles = {}

            def s_matmul(ps, j, cb, off, hi):
                """S^T matmul into ps[:, cb:cb+512] (masks applied post-exp)."""
                # widen narrow f32r matmuls to >=256 cols (4x rate penalty
                # below 256); extra cols are never read downstream
                off_mm = off if hi - off >= 256 else max(0, hi - 256)
                nc.tensor.matmul(
                    ps[:, cb + off_mm : cb + hi],
                    kT[t][:, 128 * j : 128 * (j + 1)],
                    qT[t][:, 512 * c4 + off_mm : 512 * c4 + hi],
                    start=True,
                    stop=True,
                )

            def do_exp(ps, es, j, lo, hi):
                nc.scalar.activation(
                    es[:, lo:hi],
                    ps[:, lo:hi],
                    ACTF.Exp,
                    bias=biasv_sb[:, t : t + 1],
                    scale=1.0 / (K2 * SCALE),
                )
                # multiplicative masks (exp of additive bias, incl 0 for
                # causal/local cutoffs) over the diagonal-band windows
                has_diag = j >= base
                has_pr = has_prev and base <= j + 1 <= base + 3
                if has_diag and has_pr:
                    col = 128 * (j - base)
                    nc.vector.tensor_mul(
                        es[:, col : col + 256], es[:, col : col + 256],
                        mask_sb[:, t, :, :],
                    )
                elif has_diag:
                    col = 128 * (j - base)
                    nc.vector.tensor_mul(
                        es[:, col : col + 128], es[:, col : col + 128],
                        mask_sb[:, t, 0, :],
                    )
                elif has_pr:
                    col = 128 * (j + 1 - base)
                    nc.vector.tensor_mul(
                        es[:, col : col + 128], es[:, col : col + 128],
                        mask_sb[:, t, 1, :],
                    )

            for j in range(jlo_c, base + 4):
                off = 128 * max(0, j - base)
                hi = 512 if is_causal else 128 * min(4, (j - base) + 2)
                ps = sp.tile([128, 512], f32, tag="spsum")
                es = expp.tile([128, 512], bf16, tag="es")
                s_matmul(ps, j, 0, off, hi)
                do_exp(ps, es, j, off, hi)
                es_tiles[j] = (es, 0)
            return es_tiles

        def av_phase(t, c4, es_tiles):
            """AV + gated combine for type t, query chunk c4."""
            is_causal = t < 2
            base = 4 * c4
            for i in range(base, base + 4):
                jlo_i = 0 if is_causal else max(0, i - 1)
                pav = avp.tile([128, DH + 2], f32, tag="avpsum")
                col = 128 * (i - base)
                for j in range(jlo_i, i + 1):
                    est, cb = es_tiles[j]
                    nc.tensor.matmul(
                        pav,
                        est[:, cb + col : cb + col + 128],
                        V3[:, j, t, :],
                        start=(j == jlo_i),
                        stop=(j == i),
                    )
                rc = avs.tile([128, 1], f32, tag="rcav")
                nc.vector.reciprocal(rc, pav[:, DH : DH + 1])
                if t == 0:
                    nc.vector.tensor_scalar(
                        comb[:, i, :], pav[:, 0:DH], rc, wgt[:, i, t : t + 1],
                        op0=ALU.mult, op1=ALU.mult,
                    )
                else:
                    rcw = avs.tile([128, 1], f32, tag="rcw")
                    nc.vector.tensor_mul(rcw, rc, wgt[:, i, t : t + 1])
                    dst = comb if t == 1 else comb16
                    nc.vector.scalar_tensor_tensor(
                        dst[:, i, :], pav[:, 0:DH], rcw, comb[:, i, :],
                        op0=ALU.mult, op1=ALU.add,
                    )
                    if t == 2:
                        # block finished: transpose to [dh, tok] for the exchange
                        tp = avp.tile([128, 128], bf16, tag="avpsum")
                        nc.tensor.transpose(tp, comb16[:, i, :], ident16)
                        nc.vector.tensor_copy(combT[:, i % 2, i // 2, :], tp)

        def do_rs(p):
            """ReduceScatter piece p over the core pair. rs_in rows: first 128
            = rank0's (even) blocks, next 128 = rank1's (odd) blocks."""
            base, nb = PIECES[p]
            j0 = base // 2
            for jj in range(nb // 2):
                nc.scalar.dma_start(
                    out=rs_in[p].rearrange("(hh p) (j m) -> p hh j m", hh=2, m=128)[
                        :, :, jj, :
                    ],
                    in_=combT[:, :, j0 + jj, :],
                )
            nc.gpsimd.collective_compute(
                "ReduceScatter",
                mybir.AluOpType.add,
                replica_groups=[[0, 1], [2, 3], [4, 5], [6, 7]],
                ins=[rs_in[p].opt()],
                outs=[rs_out[p].opt()],
            )

        # ---------------- main schedule: slabs interleaved with attention ----
        load_qk_weights(0)
        load_slab(0)
        load_consts()
        load_slab(1)
        for c4 in range(NCH):
            qk_part(c4)
            if c4 < 2:
                # early chunks: run S/exp (qT/kT only) while the xs stream
                # is still landing, then V-proj, then AV
                es_all = [s_phase(t, c4) for t in range(NT)]
                v_part(c4)
                if c4 + 2 < NCH:
                    load_slab(c4 + 2)
                for t in range(NT):
                    av_phase(t, c4, es_all[t])
            else:
                v_part(c4)
                if c4 + 2 < NCH:
                    load_slab(c4 + 2)
                # keep the Act exp stream fed: next type's S phase is
                # emitted before the previous type's AV phase
                es0 = s_phase(0, c4)
                es1 = s_phase(1, c4)
                av_phase(0, c4, es0)
                es2 = s_phase(2, c4)
                av_phase(1, c4, es1)
                if c4 == 3:
                    with tc.high_priority(offset=400):
                        av_phase(2, c4, es2)
                else:
                    av_phase(2, c4, es2)
            if c4 == 0:
                do_rs(0)
            elif c4 == 2:
                do_rs(1)

        # ---------------- output projection per RS piece ---------------------
        with tc.tile_pool(name="fin", bufs=6) as fin:
            if dbg:
                nc.scalar.dma_start(out=dbg_wgt[:, :, :], in_=wgt[:, :, :])

            def fin_piece(p):
                pbase, nb = PIECES[p]
                halfT = fin.tile([128, 64 * nb], bf16, tag="halfT",
                                 name=f"halfT{p}")
                nc.sync.dma_start(out=halfT, in_=rs_out[p][:, :])
                for i in range(nb // 2):  # owned blocks in this piece
                    # owned block index within full sequence: pbase + 2*i + h
                    # (h-dependence handled on the host side when scattering)
                    row0 = (pbase // 2 + i) * 128
                    for n2 in range(2):
                        nsl = slice(512 * n2, 512 * (n2 + 1))
                        pf = php.tile([128, 512], f32, tag="php")
                        nc.tensor.matmul(
                            pf,
                            halfT[:, 128 * i : 128 * (i + 1)],
                            Wo_sb[:, nsl],
                            start=True,
                            stop=True,
                        )
                        ob = fin.tile([128, 512], f32, tag="ob")
                        nc.vector.tensor_add(ob, pf, wob_sb[:, nsl])
                        eng = nc.sync if n2 == 0 else nc.scalar
                        eng.dma_start(out=out[row0 : row0 + 128, nsl], in_=ob)

            # pieces 0/1 project while the last piece's collective runs;
            # do_rs(2) is emitted after them so its long semaphore wait
            # doesn't block their queue entries
            fin_piece(0)
            fin_piece(1)
            with tc.high_priority(offset=200):
                do_rs(2)
            fin_piece(2)

    nc.compile()
    return nc


def _prep_inputs(inputs):
    """Build the 8 per-core input maps from the full problem inputs."""
    x = np.asarray(inputs["x"], dtype=np.float32)
    rel_emb = np.asarray(inputs["rel_emb"], dtype=np.float32)
    router_W = np.asarray(inputs["router_W"], dtype=np.float32)
    router_b = np.asarray(inputs["router_b"], dtype=np.float32)
    W_o = np.asarray(inputs["W_o"], dtype=np.float32)
    W_o_b = np.asarray(inputs["W_o_b"], dtype=np.float32)

    # job order is (global, rel, local) = reference type indices (1, 2, 0);
    # permute router columns so logit column t matches job t
    perm = [1, 2, 0]
    router_W = router_W[:, perm]
    router_b = router_b[perm]

    w_by_type = {
        "q": [inputs["global_Wq"], inputs["rel_Wq"], inputs["local_Wq"]],
        "k": [inputs["global_Wk"], inputs["rel_Wk"], inputs["local_Wk"]],
        "v": [inputs["global_Wv"], inputs["rel_Wv"], inputs["local_Wv"]],
    }

    p = np.arange(128)[:, None]
    q = np.arange(128)[None, :]
    tri01 = (p <= q).astype(np.float32)       # j<=i within diag block
    win01 = (p >= q).astype(np.float32)       # j>=i-128 within prev block

    f8 = ml_dtypes.float8_e4m3

    def relv(h, d):
        return rel_emb[h, np.clip(d, -MRP, MRP) + MRP]

    in_maps = []
    for c in range(8):
        b, h = c // 2, c % 2
        rel0 = float(rel_emb[h, 0])
        # multiplicative post-exp masks: exp of the additive bias, with 0 at
        # causal/local cutoffs
        m = np.zeros((NT, 2, 128, 128), np.float32)  # cast to bf16 below
        m[0, 0] = tri01
        m[1, 0] = np.exp(relv(h, p - q) - rel0) * tri01
        m[1, 1] = np.exp(relv(h, p - q - 128) - rel0)
        m[2, 0] = tri01
        m[2, 1] = win01
        bv = np.zeros((128, NT), np.float32)
        bv[:, 1] = rel0

        wq_ = np.stack(
            [np.asarray(w_by_type["q"][t][h], np.float32) for t in range(NT)]
        )
        wk_ = np.stack(
            [np.asarray(w_by_type["k"][t][h], np.float32) for t in range(NT)]
        )
        wv_ = np.concatenate(
            [np.asarray(w_by_type["v"][t][h], np.float32) * 0.5 for t in range(NT)]
            + [router_W, np.zeros((D, 1), np.float32)],
            axis=1,
        )
        in_maps.append(
            {
                "xT": np.ascontiguousarray(x[b].T),
                "x8T": np.ascontiguousarray(x[b].T).astype(f8),
                "wq": np.ascontiguousarray(wq_ * WS).astype(f8),
                "wk": np.ascontiguousarray(wk_ * WS).astype(f8),
                "wv": np.ascontiguousarray(wv_),
                "masks": m.astype(ml_dtypes.bfloat16),
                "biasv": bv,
                "bb": np.broadcast_to(router_b, (128, NT)).copy(),
                "Wo": np.ascontiguousarray(W_o).astype(ml_dtypes.bfloat16),
                "wob": np.broadcast_to(W_o_b, (128, D)).copy(),
            }
        )
    return in_maps


def kernel(**inputs) -> np.ndarray:
    from concourse.bass_utils import run_bass_kernel_spmd

    if "nc" not in _CACHE:
        _CACHE["nc"] = _build_nc()
    nc = _CACHE["nc"]

    in_maps = _prep_inputs(inputs)
    trace = os.environ.get("KERNEL_TRACE", "0") == "1"
    res = run_bass_kernel_spmd(
        nc, in_maps, core_ids=list(range(8)), trace=trace
    )
    _CACHE["last_result"] = res

    out = np.empty((B, T, D), np.float32)
    for c in range(8):
        b, h = c // 2, c % 2
        r = res.results[c]["out"].reshape(NB // 2, 128, D)
        for j in range(NB // 2):
            blk = 2 * j + h
            out[b, 128 * blk : 128 * (blk + 1), :] = r[j]
    return out


# revision 124
# speedup vs baseline: 1.0106x; 1.0106x over previous
"""Trainium2 Bass kernel for nn_MixtureOfAttentionHeads.

Sharding: 8 cores = 4 batches x 2 heads. Core c handles batch c//2, head c%2:
it computes all three attention types (global/rel/local) for its head over the
full sequence, applies the router gate per token, then pairwise ReduceScatters
sum the two heads (0.5 factor folded into Wv); each core projects its owned
token blocks with W_o and writes them to DRAM.

Main optimizations over the straightforward version:
- Q/K projections in fp8e4m3 DoubleRow mode (4 accumulation steps over the
  256-deep double rows instead of 8, at 0.5 cycles/row). Weights are
  pre-scaled x32 on the host; the 32*32=1024 score scale folds into the
  exp scale, so fp8 costs no extra on-chip ops.
- V/router projection stays f32r: top-k routing decisions flip on ~1e-3
  logit perturbations and single flips produce O(10%) point errors, so the
  router path needs near-fp32 logits.
- Rel-position bias and causal/local masks are applied MULTIPLICATIVELY
  after the exp (es *= exp(bias) tiles precomputed on host) instead of as
  additive mask matmuls - frees the PE dispatch stream.
- Interleaved block ownership (rank h owns blocks with block%2==h) so every
  ReduceScatter piece carries tokens of both ranks; 3 pieces sized (4,8,4)
  blocks fire as their chunks finish, hiding all but the last collective's
  15us fixed cost.
- Per-chunk schedule staged as S->exp (only needs Q/K) ahead of the V
  projection so the tensor engine never idles on the f32r x DMA stream, and
  late chunks interleave next-type S phases with AV phases to keep the
  activation engine's exp stream saturated.

The SPMD program is identical on every core; all per-core differences
(batch, head weights, rel-bias tiles) arrive as input data.
"""

import os
import sys

sys.path.insert(0, "/opt/trn_rl_repo")

import numpy as np
import ml_dtypes

# --- problem constants (hardcoded per contract) ---
B, T, D, DH = 4, 2048, 1024, 128
H, NT, TOPK = 2, 3, 2
WIN, MRP = 128, 32
SCALE = float(np.sqrt(DH))
NB = T // 128          # 16 query/key blocks
NCH = T // 512         # 4 query chunks of 512
WS = 32.0              # fp8 weight prescale; scores come out x(WS*WS)
K2 = WS * WS           # 1024 score scale

_CACHE = {}

# RS pieces: (first block, nblocks). Rank h owns blocks with block%2==h, so
# every piece carries tokens for both ranks of the pair.
PIECES = [(0, 4), (4, 8), (12, 4)]


def _build_nc():
    import concourse.bass as bass  # noqa: F401
    import concourse.mybir as mybir
    import concourse.tile as tile
    from concourse import bacc
    from concourse.masks import make_identity
    from contextlib import ExitStack

    dt = mybir.dt
    f32, f32r, bf16, f8 = dt.float32, dt.float32r, dt.bfloat16, dt.float8e4
    AX = mybir.AxisListType
    ALU = mybir.AluOpType
    ACTF = mybir.ActivationFunctionType
    DR = mybir.MatmulPerfMode.DoubleRow

    nc = bacc.Bacc("TRN2", target_bir_lowering=False, num_devices=8)

    xT = nc.dram_tensor("xT", [D, T], f32r, kind="ExternalInput")
    x8T = nc.dram_tensor("x8T", [D, T], f8, kind="ExternalInput")
    wq = nc.dram_tensor("wq", [NT, D, DH], f8, kind="ExternalInput")
    wk = nc.dram_tensor("wk", [NT, D, DH], f8, kind="ExternalInput")
    wv = nc.dram_tensor("wv", [D, NT * DH + NT + 1], f32r, kind="ExternalInput")
    masks = nc.dram_tensor("masks", [NT, 2, 128, 128], bf16, kind="ExternalInput")
    biasv = nc.dram_tensor("biasv", [128, NT], f32, kind="ExternalInput")
    bb = nc.dram_tensor("bb", [128, NT], f32, kind="ExternalInput")
    Wo = nc.dram_tensor("Wo", [DH, D], bf16, kind="ExternalInput")
    wob = nc.dram_tensor("wob", [128, D], f32, kind="ExternalInput")
    out = nc.dram_tensor("out", [T // 2, D], f32, kind="ExternalOutput")
    dbg = os.environ.get("KERNEL_DEBUG", "0") == "1"
    if dbg:
        dbg_wgt = nc.dram_tensor("dbg_wgt", [128, NB, NT], f32, kind="ExternalOutput")

    NV = NT * DH + NT + 1  # 388 (even, fp32r requirement)

    with tile.TileContext(nc) as tc, ExitStack() as ctx:
        persist = ctx.enter_context(tc.tile_pool(name="persist", bufs=1))
        xsp = ctx.enter_context(tc.tile_pool(name="xsp", bufs=6))
        x8p = ctx.enter_context(tc.tile_pool(name="x8p", bufs=3))
        gatep = ctx.enter_context(tc.tile_pool(name="gatep", bufs=8))
        expp = ctx.enter_context(tc.tile_pool(name="expp", bufs=44))
        avs = ctx.enter_context(tc.tile_pool(name="avs", bufs=6))
        dram = ctx.enter_context(tc.tile_pool(name="dram", bufs=1, space="DRAM"))
        php = ctx.enter_context(tc.tile_pool(name="php", bufs=2, space="PSUM"))
        phv = ctx.enter_context(tc.tile_pool(name="phv", bufs=2, space="PSUM"))
        sp = ctx.enter_context(tc.tile_pool(name="sp", bufs=2, space="PSUM"))
        avp = ctx.enter_context(tc.tile_pool(name="avp", bufs=2, space="PSUM"))

        # ---- persistent SBUF tensors ----
        qT = [persist.tile([128, T], f32r, tag=f"qT{t}", name=f"qT{t}") for t in range(NT)]
        kT = [persist.tile([128, T], f32r, tag=f"kT{t}", name=f"kT{t}") for t in range(NT)]
        V3 = persist.tile([128, NB, NT, DH + 2], bf16, tag="V3")
        wgt = persist.tile([128, NB, NT], f32, tag="wgt")
        comb = persist.tile([128, NB, DH], f32, tag="comb")
        comb16 = persist.tile([128, NB, DH], bf16, tag="comb16")
        # [dh, parity, j, tok]: block 2j+hh lives at [:, hh, j, :]
        combT = persist.tile([128, 2, NB // 2, 128], bf16, tag="combT")
        mask_sb = persist.tile([128, NT, 2, 128], bf16, tag="masks")
        biasv_sb = persist.tile([128, NT], f32, tag="biasv")
        bb_sb = persist.tile([128, NT], f32, tag="bb")
        Wo_sb = persist.tile([128, D], bf16, tag="Wo")
        wob_sb = persist.tile([128, D], f32, tag="wob")
        ident16 = persist.tile([128, 128], bf16, tag="ident16")

        wq_sb = persist.tile([128, NT, 8, DH], f8, tag="wq")
        wk_sb = persist.tile([128, NT, 8, DH], f8, tag="wk")
        wv_sb = persist.tile([128, 8, NV], f32r, tag="wv")

        def load_consts():
            # wv (big, needed early) on the scalar HWDGE queue; the other
            # constants ride the otherwise-idle SWDGE (gpsimd/Pool) path so
            # they don't contend with the x stream on the DMA engines
            nc.scalar.dma_start(
                out=wv_sb, in_=wv.rearrange("(k p) m -> p k m", p=128)
            )
            for t in range(NT):
                for kind in range(2):
                    nc.scalar.dma_start(
                        out=mask_sb[:, t, kind, :], in_=masks[t, kind, :, :]
                    )
            nc.scalar.dma_start(out=biasv_sb, in_=biasv[:, :])
            nc.scalar.dma_start(out=bb_sb, in_=bb[:, :])
            # Wo/wob are needed only by the output projection (~45us in);
            # ship them on the idle SWDGE path off the main DMA engines
            nc.gpsimd.dma_start(out=Wo_sb, in_=Wo[:, :])
            nc.gpsimd.dma_start(out=wob_sb, in_=wob[:, :])

        make_identity(nc, ident16)
        nc.vector.memset(V3[:, :, :, DH : DH + 2], 0.0)
        nc.vector.memset(V3[:, :, :, DH : DH + 1], 1.0)

        rs_in = [
            dram.tile([2 * 128, 64 * nb], bf16, name=f"rs_in{p}")
            for p, (_, nb) in enumerate(PIECES)
        ]
        rs_out = [
            dram.tile([128, 64 * nb], bf16, name=f"rs_out{p}")
            for p, (_, nb) in enumerate(PIECES)
        ]

        def load_qk_weights(t):
            nc.sync.dma_start(
                out=wq_sb[:, t, :, :],
                in_=wq[t].rearrange("(k p) m -> p k m", p=128),
            )
            nc.scalar.dma_start(
                out=wk_sb[:, t, :, :],
                in_=wk[t].rearrange("(k p) m -> p k m", p=128),
            )

        xs_tiles = {}

        x8_tiles = {}

        def load_slab(c4):
            """DMA the fp8 + f32r x tiles for token slab c4 (512 toks)."""
            sl = slice(512 * c4, 512 * (c4 + 1))
            x8t = x8p.tile([128, 8, 512], f8, tag="x8", name=f"x8_{c4}")
            for k in range(2):
                nc.sync.dma_start(
                    out=x8t[:, 4 * k : 4 * k + 4, :],
                    in_=x8T.rearrange("(k p) m -> p k m", p=128)[
                        :, 4 * k : 4 * k + 4, sl
                    ],
                )
            x8_tiles[c4] = x8t
            xs = []
            for kk in range(4):  # k-pairs: half the HWDGE queue entries
                xp2 = xsp.tile([128, 2, 512], f32r, tag="xs",
                               name=f"xs_{c4}_{kk}")
                nc.sync.dma_start(
                    out=xp2,
                    in_=xT.rearrange("(k p) m -> p k m", p=128)[
                        :, 2 * kk : 2 * kk + 2, sl
                    ],
                )
                xs.append(xp2[:, 0, :])
                xs.append(xp2[:, 1, :])
            xs_tiles[c4] = xs

        def qk_part(c4):
            """fp8 DoubleRow Q/K projections for slab c4.

            NOTE: every weight-DMA must be ISSUED before the matmuls that
            read it (slab 0 consumes all three types) — first-run reads of
            not-yet-written SBUF otherwise.
            """
            sl = slice(512 * c4, 512 * (c4 + 1))
            x8t = x8_tiles.pop(c4)
            if c4 == 0:
                load_qk_weights(1)
                load_qk_weights(2)
            for t in range(NT):
                psq = php.tile([128, 512], f32, tag="php")
                for k in range(4):
                    nc.tensor.matmul(
                        psq,
                        wq_sb[:, t, 2 * k : 2 * k + 2, :],
                        x8t[:, 2 * k : 2 * k + 2, :],
                        start=(k == 0), stop=(k == 3),
                        perf_mode=DR,
                    )
                nc.scalar.copy(qT[t][:, sl], psq)
                psk = php.tile([128, 512], f32, tag="php")
                for k in range(4):
                    nc.tensor.matmul(
                        psk,
                        wk_sb[:, t, 2 * k : 2 * k + 2, :],
                        x8t[:, 2 * k : 2 * k + 2, :],
                        start=(k == 0), stop=(k == 3),
                        perf_mode=DR,
                    )
                nc.vector.tensor_copy(kT[t][:, sl], psk)

        def v_part(c4):
            """V + router projection and gates for slab c4 (needs xs)."""
            xs = xs_tiles.pop(c4)
            lg4 = gatep.tile([128, 4, NT], f32, tag="lg4")
            for ib in range(4):
                i = 4 * c4 + ib
                ibo = 128 * ib
                psv = phv.tile([128, NV], f32, tag="phv")
                for k in range(8):
                    nc.tensor.matmul(
                        psv,
                        xs[k][:, ibo : ibo + 128],
                        wv_sb[:, k, :],
                        start=(k == 0),
                        stop=(k == 7),
                    )
                nc.vector.tensor_copy(V3[:, i, :, 0:DH], psv[:, 0 : NT * DH])
                nc.vector.tensor_add(
                    lg4[:, ib, :], psv[:, NT * DH : NT * DH + NT], bb_sb
                )
            # vectorized gate pipeline over the whole slab (4 blocks at once)
            mn = gatep.tile([128, 4, 1], f32, tag="mn")
            nc.vector.tensor_tensor(
                mn, lg4[:, :, 0:1], lg4[:, :, 1:2], op=ALU.min
            )
            mn2 = gatep.tile([128, 4, 1], f32, tag="mn2")
            nc.vector.tensor_tensor(mn2, mn, lg4[:, :, 2:3], op=ALU.min)
            eg = gatep.tile([128, 4, NT], f32, tag="eg")
            nc.scalar.activation(eg, lg4, ACTF.Exp)
            gt = gatep.tile([128, 4, NT], f32, tag="gt")
            nc.vector.tensor_tensor(
                gt, lg4, mn2.broadcast_to([128, 4, NT]), op=ALU.is_gt
            )
            ew = gatep.tile([128, 4, NT], f32, tag="ew")
            nc.vector.tensor_mul(ew, eg, gt)
            sm = gatep.tile([128, 4, 1], f32, tag="sm")
            nc.vector.tensor_reduce(sm, ew, axis=AX.X, op=ALU.add)
            rc = gatep.tile([128, 4, 1], f32, tag="rc")
            nc.vector.reciprocal(rc, sm)
            nc.vector.tensor_mul(
                wgt[:, 4 * c4 : 4 * c4 + 4, :], ew, rc.broadcast_to([128, 4, NT])
            )

        def s_phase(t, c4):
            """S^T -> exp -> masked es tiles for type t, query chunk c4."""
            is_causal = t < 2
            has_prev = t >= 1
            base = 4 * c4
            jlo_c = 0 if is_causal else max(0, base - 1)
            es_tiles = {}

            def s_matmul(ps, j, cb, off, hi):
                """S^T matmul into ps[:, cb:cb+512] (masks applied post-exp)."""
                # widen narrow f32r matmuls to >=256 cols (4x rate penalty
                # below 256); extra cols are never read downstream
                off_mm = off if hi - off >= 256 else max(0, hi - 256)
                nc.tensor.matmul(
                    ps[:, cb + off_mm : cb + hi],
                    kT[t][:, 128 * j : 128 * (j + 1)],
                    qT[t][:, 512 * c4 + off_mm : 512 * c4 + hi],
                    start=True,
                    stop=True,
                )

            def do_exp(ps, es, j, lo, hi):
                nc.scalar.activation(
                    es[:, lo:hi],
                    ps[:, lo:hi],
                    ACTF.Exp,
                    bias=biasv_sb[:, t : t + 1],
                    scale=1.0 / (K2 * SCALE),
                )
                # multiplicative masks (exp of additive bias, incl 0 for
                # causal/local cutoffs) over the diagonal-band windows
                has_diag = j >= base
                has_pr = has_prev and base <= j + 1 <= base + 3
                if has_diag and has_pr:
                    col = 128 * (j - base)
                    nc.vector.tensor_mul(
                        es[:, col : col + 256], es[:, col : col + 256],
                        mask_sb[:, t, :, :],
                    )
                elif has_diag:
                    col = 128 * (j - base)
                    nc.vector.tensor_mul(
                        es[:, col : col + 128], es[:, col : col + 128],
                        mask_sb[:, t, 0, :],
                    )
                elif has_pr:
                    col = 128 * (j + 1 - base)
                    nc.vector.tensor_mul(
                        es[:, col : col + 128], es[:, col : col + 128],
                        mask_sb[:, t, 1, :],
                    )

            for j in range(jlo_c, base + 4):
                off = 128 * max(0, j - base)
                hi = 512 if is_causal else 128 * min(4, (j - base) + 2)
                ps = sp.tile([128, 512], f32, tag="spsum")
                es = expp.tile([128, 512], bf16, tag="es")
                s_matmul(ps, j, 0, off, hi)
                do_exp(ps, es, j, off, hi)
                es_tiles[j] = (es, 0)
            return es_tiles

        def av_phase(t, c4, es_tiles):
            """AV + gated combine for type t, query chunk c4."""
            is_causal = t < 2
            base = 4 * c4
            for i in range(base, base + 4):
                jlo_i = 0 if is_causal else max(0, i - 1)
                pav = avp.tile([128, DH + 2], f32, tag="avpsum")
                col = 128 * (i - base)
                for j in range(jlo_i, i + 1):
                    est, cb = es_tiles[j]
                    nc.tensor.matmul(
                        pav,
                        est[:, cb + col : cb + col + 128],
                        V3[:, j, t, :],
                        start=(j == jlo_i),
                        stop=(j == i),
                    )
                rc = avs.tile([128, 1], f32, tag="rcav")
                nc.vector.reciprocal(rc, pav[:, DH : DH + 1])
                if t == 0:
                    nc.vector.tensor_scalar(
                        comb[:, i, :], pav[:, 0:DH], rc, wgt[:, i, t : t + 1],
                        op0=ALU.mult, op1=ALU.mult,
                    )
                else:
                    rcw = avs.tile([128, 1], f32, tag="rcw")
                    nc.vector.tensor_mul(rcw, rc, wgt[:, i, t : t + 1])
                    dst = comb if t == 1 else comb16
                    nc.vector.scalar_tensor_tensor(
                        dst[:, i, :], pav[:, 0:DH], rcw, comb[:, i, :],
                        op0=ALU.mult, op1=ALU.add,
                    )
                    if t == 2:
                        # block finished: transpose to [dh, tok] for the exchange
                        tp = avp.tile([128, 128], bf16, tag="avpsum")
                        nc.tensor.transpose(tp, comb16[:, i, :], ident16)
                        nc.vector.tensor_copy(combT[:, i % 2, i // 2, :], tp)

        def do_rs(p):
            """ReduceScatter piece p over the core pair. rs_in rows: first 128
            = rank0's (even) blocks, next 128 = rank1's (odd) blocks."""
            base, nb = PIECES[p]
            j0 = base // 2
            for jj in range(nb // 2):
                nc.scalar.dma_start(
                    out=rs_in[p].rearrange("(hh p) (j m) -> p hh j m", hh=2, m=128)[
                        :, :, jj, :
                    ],
                    in_=combT[:, :, j0 + jj, :],
                )
            nc.gpsimd.collective_compute(
                "ReduceScatter",
                mybir.AluOpType.add,
                replica_groups=[[0, 1], [2, 3], [4, 5], [6, 7]],
                ins=[rs_in[p].opt()],
                outs=[rs_out[p].opt()],
            )

        # ---------------- main schedule: slabs interleaved with attention ----
        load_qk_weights(0)
        load_slab(0)
        load_consts()
        load_slab(1)
        for c4 in range(NCH):
            qk_part(c4)
            if c4 < 2:
                # early chunks: run S/exp (qT/kT only) while the xs stream
                # is still landing, then V-proj, then AV
                es_all = [s_phase(t, c4) for t in range(NT)]
                v_part(c4)
                if c4 + 2 < NCH:
                    load_slab(c4 + 2)
                for t in range(NT):
                    av_phase(t, c4, es_all[t])
            else:
                v_part(c4)
                if c4 + 2 < NCH:
                    load_slab(c4 + 2)
                # keep the Act exp stream fed: next type's S phase is
                # emitted before the previous type's AV phase
                es0 = s_phase(0, c4)
                es1 = s_phase(1, c4)
                av_phase(0, c4, es0)
                es2 = s_phase(2, c4)
                av_phase(1, c4, es1)
                if c4 == 3:
                    with tc.high_priority(offset=400):
                        av_phase(2, c4, es2)
                else:
                    av_phase(2, c4, es2)
            if c4 == 0:
                do_rs(0)
            elif c4 == 2:
                do_rs(1)

        # ---------------- output projection per RS piece ---------------------
        with tc.tile_pool(name="fin", bufs=6) as fin:
            if dbg:
                nc.scalar.dma_start(out=dbg_wgt[:, :, :], in_=wgt[:, :, :])

            def fin_piece(p):
                pbase, nb = PIECES[p]
                halfT = fin.tile([128, 64 * nb], bf16, tag="halfT",
                                 name=f"halfT{p}")
                nc.sync.dma_start(out=halfT, in_=rs_out[p][:, :])
                for i in range(nb // 2):  # owned blocks in this piece
                    # owned block index within full sequence: pbase + 2*i + h
                    # (h-dependence handled on the host side when scattering)
                    row0 = (pbase // 2 + i) * 128
                    for n2 in range(2):
                        nsl = slice(512 * n2, 512 * (n2 + 1))
                        pf = php.tile([128, 512], f32, tag="php")
                        nc.tensor.matmul(
                            pf,
                            halfT[:, 128 * i : 128 * (i + 1)],
                            Wo_sb[:, nsl],
                            start=True,
                            stop=True,
                        )
                        ob = fin.tile([128, 512], f32, tag="ob")
                        nc.vector.tensor_add(ob, pf, wob_sb[:, nsl])
                        eng = nc.sync if n2 == 0 else nc.scalar
                        eng.dma_start(out=out[row0 : row0 + 128, nsl], in_=ob)

            # pieces 0/1 project while the last piece's collective runs;
            # do_rs(2) is emitted after them so its long semaphore wait
            # doesn't block their queue entries
            fin_piece(0)
            fin_piece(1)
            with tc.high_priority(offset=200):
                do_rs(2)
            fin_piece(2)

    nc.compile()
    return nc


def _prep_inputs(inputs):
    """Build the 8 per-core input maps from the full problem inputs."""
    x = np.asarray(inputs["x"], dtype=np.float32)
    rel_emb = np.asarray(inputs["rel_emb"], dtype=np.float32)
    router_W = np.asarray(inputs["router_W"], dtype=np.float32)
    router_b = np.asarray(inputs["router_b"], dtype=np.float32)
    W_o = np.asarray(inputs["W_o"], dtype=np.float32)
    W_o_b = np.asarray(inputs["W_o_b"], dtype=np.float32)

    # job order is (global, rel, local) = reference type indices (1, 2, 0);
    # permute router columns so logit column t matches job t
    perm = [1, 2, 0]
    router_W = router_W[:, perm]
    router_b = router_b[perm]

    w_by_type = {
        "q": [inputs["global_Wq"], inputs["rel_Wq"], inputs["local_Wq"]],
        "k": [inputs["global_Wk"], inputs["rel_Wk"], inputs["local_Wk"]],
        "v": [inputs["global_Wv"], inputs["rel_Wv"], inputs["local_Wv"]],
    }

    p = np.arange(128)[:, None]
    q = np.arange(128)[None, :]
    tri01 = (p <= q).astype(np.float32)       # j<=i within diag block
    win01 = (p >= q).astype(np.float32)       # j>=i-128 within prev block

    f8 = ml_dtypes.float8_e4m3

    def relv(h, d):
        return rel_emb[h, np.clip(d, -MRP, MRP) + MRP]

    in_maps = []
    for c in range(8):
        b, h = c // 2, c % 2
        rel0 = float(rel_emb[h, 0])
        # multiplicative post-exp masks: exp of the additive bias, with 0 at
        # causal/local cutoffs
        m = np.zeros((NT, 2, 128, 128), np.float32)  # cast to bf16 below
        m[0, 0] = tri01
        m[1, 0] = np.exp(relv(h, p - q) - rel0) * tri01
        m[1, 1] = np.exp(relv(h, p - q - 128) - rel0)
        m[2, 0] = tri01
        m[2, 1] = win01
        bv = np.zeros((128, NT), np.float32)
        bv[:, 1] = rel0

        wq_ = np.stack(
            [np.asarray(w_by_type["q"][t][h], np.float32) for t in range(NT)]
        )
        wk_ = np.stack(
            [np.asarray(w_by_type["k"][t][h], np.float32) for t in range(NT)]
        )
        wv_ = np.concatenate(
            [np.asarray(w_by_type["v"][t][h], np.float32) * 0.5 for t in range(NT)]
            + [router_W, np.zeros((D, 1), np.float32)],
            axis=1,
        )
        in_maps.append(
            {
                "xT": np.ascontiguousarray(x[b].T),
                "x8T": np.ascontiguousarray(x[b].T).astype(f8),
                "wq": np.ascontiguousarray(wq_ * WS).astype(f8),
                "wk": np.ascontiguousarray(wk_ * WS).astype(f8),
                "wv": np.ascontiguousarray(wv_),
                "masks": m.astype(ml_dtypes.bfloat16),
                "biasv": bv,
                "bb": np.broadcast_to(router_b, (128, NT)).copy(),
                "Wo": np.ascontiguousarray(W_o).astype(ml_dtypes.bfloat16),
                "wob": np.broadcast_to(W_o_b, (128, D)).copy(),
            }
        )
    return in_maps


def kernel(**inputs) -> np.ndarray:
    from concourse.bass_utils import run_bass_kernel_spmd

    if "nc" not in _CACHE:
        _CACHE["nc"] = _build_nc()
    nc = _CACHE["nc"]

    in_maps = _prep_inputs(inputs)
    trace = os.environ.get("KERNEL_TRACE", "0") == "1"
    res = run_bass_kernel_spmd(
        nc, in_maps, core_ids=list(range(8)), trace=trace
    )
    _CACHE["last_result"] = res

    out = np.empty((B, T, D), np.float32)
    for c in range(8):
        b, h = c // 2, c % 2
        r = res.results[c]["out"].reshape(NB // 2, 128, D)
        for j in range(NB // 2):
            blk = 2 * j + h
            out[b, 128 * blk : 128 * (blk + 1), :] = r[j]
    return out


# revision 127
# speedup vs baseline: 1.0114x; 1.0008x over previous
"""Trainium2 Bass kernel for nn_MixtureOfAttentionHeads.

Sharding: 8 cores = 4 batches x 2 heads. Core c handles batch c//2, head c%2:
it computes all three attention types (global/rel/local) for its head over the
full sequence, applies the router gate per token, then pairwise ReduceScatters
sum the two heads (0.5 factor folded into Wv); each core projects its owned
token blocks with W_o and writes them to DRAM.

Main optimizations over the straightforward version:
- Q/K projections in fp8e4m3 DoubleRow mode (4 accumulation steps over the
  256-deep double rows instead of 8, at 0.5 cycles/row). Weights are
  pre-scaled x32 on the host; the 32*32=1024 score scale folds into the
  exp scale, so fp8 costs no extra on-chip ops.
- V/router projection stays f32r: top-k routing decisions flip on ~1e-3
  logit perturbations and single flips produce O(10%) point errors, so the
  router path needs near-fp32 logits.
- Rel-position bias and causal/local masks are applied MULTIPLICATIVELY
  after the exp (es *= exp(bias) tiles precomputed on host) instead of as
  additive mask matmuls - frees the PE dispatch stream.
- Interleaved block ownership (rank h owns blocks with block%2==h) so every
  ReduceScatter piece carries tokens of both ranks; 3 pieces sized (4,8,4)
  blocks fire as their chunks finish, hiding all but the last collective's
  15us fixed cost.
- Per-chunk schedule staged as S->exp (only needs Q/K) ahead of the V
  projection so the tensor engine never idles on the f32r x DMA stream, and
  late chunks interleave next-type S phases with AV phases to keep the
  activation engine's exp stream saturated.

The SPMD program is identical on every core; all per-core differences
(batch, head weights, rel-bias tiles) arrive as input data.
"""

import os
import sys

sys.path.insert(0, "/opt/trn_rl_repo")

import numpy as np
import ml_dtypes

# --- problem constants (hardcoded per contract) ---
B, T, D, DH = 4, 2048, 1024, 128
H, NT, TOPK = 2, 3, 2
WIN, MRP = 128, 32
SCALE = float(np.sqrt(DH))
NB = T // 128          # 16 query/key blocks
NCH = T // 512         # 4 query chunks of 512
WS = 32.0              # fp8 weight prescale; scores come out x(WS*WS)
K2 = WS * WS           # 1024 score scale

_CACHE = {}

# RS pieces: (first block, nblocks). Rank h owns blocks with block%2==h, so
# every piece carries tokens for both ranks of the pair.
PIECES = [(0, 4), (4, 8), (12, 4)]


def _build_nc():
    import concourse.bass as bass  # noqa: F401
    import concourse.mybir as mybir
    import concourse.tile as tile
    from concourse import bacc
    from concourse.masks import make_identity
    from contextlib import ExitStack

    dt = mybir.dt
    f32, f32r, bf16, f8 = dt.float32, dt.float32r, dt.bfloat16, dt.float8e4
    AX = mybir.AxisListType
    ALU = mybir.AluOpType
    ACTF = mybir.ActivationFunctionType
    DR = mybir.MatmulPerfMode.DoubleRow

    nc = bacc.Bacc("TRN2", target_bir_lowering=False, num_devices=8)

    xT = nc.dram_tensor("xT", [D, T], f32r, kind="ExternalInput")
    x8T = nc.dram_tensor("x8T", [D, T], f8, kind="ExternalInput")
    wq = nc.dram_tensor("wq", [NT, D, DH], f8, kind="ExternalInput")
    wk = nc.dram_tensor("wk", [NT, D, DH], f8, kind="ExternalInput")
    wv = nc.dram_tensor("wv", [D, NT * DH + NT + 1], f32r, kind="ExternalInput")
    masks = nc.dram_tensor("masks", [NT, 2, 128, 128], bf16, kind="ExternalInput")
    biasv = nc.dram_tensor("biasv", [128, NT], f32, kind="ExternalInput")
    bb = nc.dram_tensor("bb", [128, NT], f32, kind="ExternalInput")
    Wo = nc.dram_tensor("Wo", [DH, D], bf16, kind="ExternalInput")
    wob = nc.dram_tensor("wob", [128, D], f32, kind="ExternalInput")
    out = nc.dram_tensor("out", [T // 2, D], f32, kind="ExternalOutput")
    dbg = os.environ.get("KERNEL_DEBUG", "0") == "1"
    if dbg:
        dbg_wgt = nc.dram_tensor("dbg_wgt", [128, NB, NT], f32, kind="ExternalOutput")

    NV = NT * DH + NT + 1  # 388 (even, fp32r requirement)

    with tile.TileContext(nc) as tc, ExitStack() as ctx:
        persist = ctx.enter_context(tc.tile_pool(name="persist", bufs=1))
        xsp = ctx.enter_context(tc.tile_pool(name="xsp", bufs=6))
        x8p = ctx.enter_context(tc.tile_pool(name="x8p", bufs=3))
        gatep = ctx.enter_context(tc.tile_pool(name="gatep", bufs=8))
        expp = ctx.enter_context(tc.tile_pool(name="expp", bufs=44))
        avs = ctx.enter_context(tc.tile_pool(name="avs", bufs=6))
        dram = ctx.enter_context(tc.tile_pool(name="dram", bufs=1, space="DRAM"))
        php = ctx.enter_context(tc.tile_pool(name="php", bufs=2, space="PSUM"))
        phv = ctx.enter_context(tc.tile_pool(name="phv", bufs=2, space="PSUM"))
        sp = ctx.enter_context(tc.tile_pool(name="sp", bufs=2, space="PSUM"))
        avp = ctx.enter_context(tc.tile_pool(name="avp", bufs=2, space="PSUM"))

        # ---- persistent SBUF tensors ----
        qT = [persist.tile([128, T], f32r, tag=f"qT{t}", name=f"qT{t}") for t in range(NT)]
        kT = [persist.tile([128, T], f32r, tag=f"kT{t}", name=f"kT{t}") for t in range(NT)]
        V3 = persist.tile([128, NB, NT, DH + 2], bf16, tag="V3")
        wgt = persist.tile([128, NB, NT], f32, tag="wgt")
        comb = persist.tile([128, NB, DH], f32, tag="comb")
        comb16 = persist.tile([128, NB, DH], bf16, tag="comb16")
        # [dh, parity, j, tok]: block 2j+hh lives at [:, hh, j, :]
        combT = persist.tile([128, 2, NB // 2, 128], bf16, tag="combT")
        mask_sb = persist.tile([128, NT, 2, 128], bf16, tag="masks")
        biasv_sb = persist.tile([128, NT], f32, tag="biasv")
        bb_sb = persist.tile([128, NT], f32, tag="bb")
        Wo_sb = persist.tile([128, D], bf16, tag="Wo")
        wob_sb = persist.tile([128, D], f32, tag="wob")
        ident16 = persist.tile([128, 128], bf16, tag="ident16")

        wq_sb = persist.tile([128, NT, 8, DH], f8, tag="wq")
        wk_sb = persist.tile([128, NT, 8, DH], f8, tag="wk")
        wv_sb = persist.tile([128, 8, NV], f32r, tag="wv")

        def load_consts():
            # wv (big, needed early) on the scalar HWDGE queue; the other
            # constants ride the otherwise-idle SWDGE (gpsimd/Pool) path so
            # they don't contend with the x stream on the DMA engines
            nc.scalar.dma_start(
                out=wv_sb, in_=wv.rearrange("(k p) m -> p k m", p=128)
            )
            nc.scalar.dma_start(
                out=mask_sb, in_=masks.rearrange("t k p m -> p t k m")
            )
            nc.scalar.dma_start(out=biasv_sb, in_=biasv[:, :])
            nc.scalar.dma_start(out=bb_sb, in_=bb[:, :])
            # Wo/wob are needed only by the output projection (~45us in);
            # ship them on the idle SWDGE path off the main DMA engines
            nc.gpsimd.dma_start(out=Wo_sb, in_=Wo[:, :])
            nc.gpsimd.dma_start(out=wob_sb, in_=wob[:, :])

        make_identity(nc, ident16)
        nc.vector.memset(V3[:, :, :, DH : DH + 2], 0.0)
        nc.vector.memset(V3[:, :, :, DH : DH + 1], 1.0)

        rs_in = [
            dram.tile([2 * 128, 64 * nb], bf16, name=f"rs_in{p}")
            for p, (_, nb) in enumerate(PIECES)
        ]
        rs_out = [
            dram.tile([128, 64 * nb], bf16, name=f"rs_out{p}")
            for p, (_, nb) in enumerate(PIECES)
        ]

        def load_qk_weights_all():
            nc.sync.dma_start(
                out=wq_sb,
                in_=wq.rearrange("t (k p) m -> p t k m", p=128),
            )
            nc.scalar.dma_start(
                out=wk_sb,
                in_=wk.rearrange("t (k p) m -> p t k m", p=128),
            )

        xs_tiles = {}

        x8_tiles = {}

        def load_slab(c4):
            """DMA the fp8 + f32r x tiles for token slab c4 (512 toks)."""
            sl = slice(512 * c4, 512 * (c4 + 1))
            x8t = x8p.tile([128, 8, 512], f8, tag="x8", name=f"x8_{c4}")
            nc.sync.dma_start(
                out=x8t,
                in_=x8T.rearrange("(k p) m -> p k m", p=128)[:, :, sl],
            )
            x8_tiles[c4] = x8t
            xs = []
            for kk in range(4):  # k-pairs: half the HWDGE queue entries
                xp2 = xsp.tile([128, 2, 512], f32r, tag="xs",
                               name=f"xs_{c4}_{kk}")
                nc.sync.dma_start(
                    out=xp2,
                    in_=xT.rearrange("(k p) m -> p k m", p=128)[
                        :, 2 * kk : 2 * kk + 2, sl
                    ],
                )
                xs.append(xp2[:, 0, :])
                xs.append(xp2[:, 1, :])
            xs_tiles[c4] = xs

        def qk_part(c4):
            """fp8 DoubleRow Q/K projections for slab c4.

            NOTE: every weight-DMA must be ISSUED before the matmuls that
            read it (slab 0 consumes all three types) — first-run reads of
            not-yet-written SBUF otherwise.
            """
            sl = slice(512 * c4, 512 * (c4 + 1))
            x8t = x8_tiles.pop(c4)
            for t in range(NT):
                psq = php.tile([128, 512], f32, tag="php")
                for k in range(4):
                    nc.tensor.matmul(
                        psq,
                        wq_sb[:, t, 2 * k : 2 * k + 2, :],
                        x8t[:, 2 * k : 2 * k + 2, :],
                        start=(k == 0), stop=(k == 3),
                        perf_mode=DR,
                    )
                nc.scalar.copy(qT[t][:, sl], psq)
                psk = php.tile([128, 512], f32, tag="php")
                for k in range(4):
                    nc.tensor.matmul(
                        psk,
                        wk_sb[:, t, 2 * k : 2 * k + 2, :],
                        x8t[:, 2 * k : 2 * k + 2, :],
                        start=(k == 0), stop=(k == 3),
                        perf_mode=DR,
                    )
                nc.vector.tensor_copy(kT[t][:, sl], psk)

        def v_part(c4):
            """V + router projection and gates for slab c4 (needs xs)."""
            xs = xs_tiles.pop(c4)
            lg4 = gatep.tile([128, 4, NT], f32, tag="lg4")
            for ib in range(4):
                i = 4 * c4 + ib
                ibo = 128 * ib
                psv = phv.tile([128, NV], f32, tag="phv")
                for k in range(8):
                    nc.tensor.matmul(
                        psv,
                        xs[k][:, ibo : ibo + 128],
                        wv_sb[:, k, :],
                        start=(k == 0),
                        stop=(k == 7),
                    )
                nc.vector.tensor_copy(V3[:, i, :, 0:DH], psv[:, 0 : NT * DH])
                nc.vector.tensor_add(
                    lg4[:, ib, :], psv[:, NT * DH : NT * DH + NT], bb_sb
                )
            # vectorized gate pipeline over the whole slab (4 blocks at once)
            mn = gatep.tile([128, 4, 1], f32, tag="mn")
            nc.vector.tensor_tensor(
                mn, lg4[:, :, 0:1], lg4[:, :, 1:2], op=ALU.min
            )
            mn2 = gatep.tile([128, 4, 1], f32, tag="mn2")
            nc.vector.tensor_tensor(mn2, mn, lg4[:, :, 2:3], op=ALU.min)
            eg = gatep.tile([128, 4, NT], f32, tag="eg")
            nc.scalar.activation(eg, lg4, ACTF.Exp)
            gt = gatep.tile([128, 4, NT], f32, tag="gt")
            nc.vector.tensor_tensor(
                gt, lg4, mn2.broadcast_to([128, 4, NT]), op=ALU.is_gt
            )
            ew = gatep.tile([128, 4, NT], f32, tag="ew")
            nc.vector.tensor_mul(ew, eg, gt)
            sm = gatep.tile([128, 4, 1], f32, tag="sm")
            nc.vector.tensor_reduce(sm, ew, axis=AX.X, op=ALU.add)
            rc = gatep.tile([128, 4, 1], f32, tag="rc")
            nc.vector.reciprocal(rc, sm)
            nc.vector.tensor_mul(
                wgt[:, 4 * c4 : 4 * c4 + 4, :], ew, rc.broadcast_to([128, 4, NT])
            )

        def s_phase(t, c4):
            """S^T -> exp -> masked es tiles for type t, query chunk c4."""
            is_causal = t < 2
            has_prev = t >= 1
            base = 4 * c4
            jlo_c = 0 if is_causal else max(0, base - 1)
            es_tiles = {}

            def s_matmul(ps, j, cb, off, hi):
                """S^T matmul into ps[:, cb:cb+512] (masks applied post-exp)."""
                # widen narrow f32r matmuls to >=256 cols (4x rate penalty
                # below 256); extra cols are never read downstream
                off_mm = off if hi - off >= 256 else max(0, hi - 256)
                nc.tensor.matmul(
                    ps[:, cb + off_mm : cb + hi],
                    kT[t][:, 128 * j : 128 * (j + 1)],
                    qT[t][:, 512 * c4 + off_mm : 512 * c4 + hi],
                    start=True,
                    stop=True,
                )

            def do_exp(ps, es, j, lo, hi):
                nc.scalar.activation(
                    es[:, lo:hi],
                    ps[:, lo:hi],
                    ACTF.Exp,
                    bias=biasv_sb[:, t : t + 1],
                    scale=1.0 / (K2 * SCALE),
                )
                # multiplicative masks (exp of additive bias, incl 0 for
                # causal/local cutoffs) over the diagonal-band windows
                has_diag = j >= base
                has_pr = has_prev and base <= j + 1 <= base + 3
                if has_diag and has_pr:
                    col = 128 * (j - base)
                    nc.vector.tensor_mul(
                        es[:, col : col + 256], es[:, col : col + 256],
                        mask_sb[:, t, :, :],
                    )
                elif has_diag:
                    col = 128 * (j - base)
                    nc.vector.tensor_mul(
                        es[:, col : col + 128], es[:, col : col + 128],
                        mask_sb[:, t, 0, :],
                    )
                elif has_pr:
                    col = 128 * (j + 1 - base)
                    nc.vector.tensor_mul(
                        es[:, col : col + 128], es[:, col : col + 128],
                        mask_sb[:, t, 1, :],
                    )

            for j in range(jlo_c, base + 4):
                off = 128 * max(0, j - base)
                hi = 512 if is_causal else 128 * min(4, (j - base) + 2)
                ps = sp.tile([128, 512], f32, tag="spsum")
                es = expp.tile([128, 512], bf16, tag="es")
                s_matmul(ps, j, 0, off, hi)
                do_exp(ps, es, j, off, hi)
                es_tiles[j] = (es, 0)
            return es_tiles

        def av_phase(t, c4, es_tiles):
            """AV + gated combine for type t, query chunk c4."""
            is_causal = t < 2
            base = 4 * c4
            for i in range(base, base + 4):
                jlo_i = 0 if is_causal else max(0, i - 1)
                pav = avp.tile([128, DH + 2], f32, tag="avpsum")
                col = 128 * (i - base)
                for j in range(jlo_i, i + 1):
                    est, cb = es_tiles[j]
                    nc.tensor.matmul(
                        pav,
                        est[:, cb + col : cb + col + 128],
                        V3[:, j, t, :],
                        start=(j == jlo_i),
                        stop=(j == i),
                    )
                rc = avs.tile([128, 1], f32, tag="rcav")
                nc.vector.reciprocal(rc, pav[:, DH : DH + 1])
                if t == 0:
                    nc.vector.tensor_scalar(
                        comb[:, i, :], pav[:, 0:DH], rc, wgt[:, i, t : t + 1],
                        op0=ALU.mult, op1=ALU.mult,
                    )
                else:
                    rcw = avs.tile([128, 1], f32, tag="rcw")
                    nc.vector.tensor_mul(rcw, rc, wgt[:, i, t : t + 1])
                    dst = comb if t == 1 else comb16
                    nc.vector.scalar_tensor_tensor(
                        dst[:, i, :], pav[:, 0:DH], rcw, comb[:, i, :],
                        op0=ALU.mult, op1=ALU.add,
                    )
                    if t == 2:
                        # block finished: transpose to [dh, tok] for the exchange
                        tp = avp.tile([128, 128], bf16, tag="avpsum")
                        nc.tensor.transpose(tp, comb16[:, i, :], ident16)
                        nc.vector.tensor_copy(combT[:, i % 2, i // 2, :], tp)

        def do_rs(p):
            """ReduceScatter piece p over the core pair. rs_in rows: first 128
            = rank0's (even) blocks, next 128 = rank1's (odd) blocks."""
            base, nb = PIECES[p]
            j0 = base // 2
            for jj in range(nb // 2):
                nc.scalar.dma_start(
                    out=rs_in[p].rearrange("(hh p) (j m) -> p hh j m", hh=2, m=128)[
                        :, :, jj, :
                    ],
                    in_=combT[:, :, j0 + jj, :],
                )
            nc.gpsimd.collective_compute(
                "ReduceScatter",
                mybir.AluOpType.add,
                replica_groups=[[0, 1], [2, 3], [4, 5], [6, 7]],
                ins=[rs_in[p].opt()],
                outs=[rs_out[p].opt()],
            )

        # ---------------- main schedule: slabs interleaved with attention ----
        load_qk_weights_all()
        load_slab(0)
        load_consts()
        load_slab(1)
        for c4 in range(NCH):
            qk_part(c4)
            if c4 < 2:
                # early chunks: run S/exp (qT/kT only) while the xs stream
                # is still landing, then V-proj, then AV
                es_all = [s_phase(t, c4) for t in range(NT)]
                v_part(c4)
                if c4 + 2 < NCH:
                    load_slab(c4 + 2)
                for t in range(NT):
                    av_phase(t, c4, es_all[t])
            else:
                v_part(c4)
                if c4 + 2 < NCH:
                    load_slab(c4 + 2)
                # keep the Act exp stream fed: next type's S phase is
                # emitted before the previous type's AV phase
                es0 = s_phase(0, c4)
                es1 = s_phase(1, c4)
                av_phase(0, c4, es0)
                es2 = s_phase(2, c4)
                av_phase(1, c4, es1)
                if c4 == 3:
                    with tc.high_priority(offset=400):
                        av_phase(2, c4, es2)
                else:
                    av_phase(2, c4, es2)
            if c4 == 0:
                do_rs(0)
            elif c4 == 2:
                do_rs(1)

        # ---------------- output projection per RS piece ---------------------
        with tc.tile_pool(name="fin", bufs=6) as fin:
            if dbg:
                nc.scalar.dma_start(out=dbg_wgt[:, :, :], in_=wgt[:, :, :])

            def fin_piece(p):
                pbase, nb = PIECES[p]
                halfT = fin.tile([128, 64 * nb], bf16, tag="halfT",
                                 name=f"halfT{p}")
                nc.sync.dma_start(out=halfT, in_=rs_out[p][:, :])
                for i in range(nb // 2):  # owned blocks in this piece
                    # owned block index within full sequence: pbase + 2*i + h
                    # (h-dependence handled on the host side when scattering)
                    row0 = (pbase // 2 + i) * 128
                    for n2 in range(2):
                        nsl = slice(512 * n2, 512 * (n2 + 1))
                        pf = php.tile([128, 512], f32, tag="php")
                        nc.tensor.matmul(
                            pf,
                            halfT[:, 128 * i : 128 * (i + 1)],
                            Wo_sb[:, nsl],
                            start=True,
                            stop=True,
                        )
                        ob = fin.tile([128, 512], f32, tag="ob")
                        nc.vector.tensor_add(ob, pf, wob_sb[:, nsl])
                        eng = nc.sync if n2 == 0 else nc.scalar
                        eng.dma_start(out=out[row0 : row0 + 128, nsl], in_=ob)

            # pieces 0/1 project while the last piece's collective runs;
            # do_rs(2) is emitted after them so its long semaphore wait
            # doesn't block their queue entries
            fin_piece(0)
            fin_piece(1)
            with tc.high_priority(offset=200):
                do_rs(2)
            fin_piece(2)

    nc.compile()
    return nc


def _prep_inputs(inputs):
    """Build the 8 per-core input maps from the full problem inputs."""
    x = np.asarray(inputs["x"], dtype=np.float32)
    rel_emb = np.asarray(inputs["rel_emb"], dtype=np.float32)
    router_W = np.asarray(inputs["router_W"], dtype=np.float32)
    router_b = np.asarray(inputs["router_b"], dtype=np.float32)
    W_o = np.asarray(inputs["W_o"], dtype=np.float32)
    W_o_b = np.asarray(inputs["W_o_b"], dtype=np.float32)

    # job order is (global, rel, local) = reference type indices (1, 2, 0);
    # permute router columns so logit column t matches job t
    perm = [1, 2, 0]
    router_W = router_W[:, perm]
    router_b = router_b[perm]

    w_by_type = {
        "q": [inputs["global_Wq"], inputs["rel_Wq"], inputs["local_Wq"]],
        "k": [inputs["global_Wk"], inputs["rel_Wk"], inputs["local_Wk"]],
        "v": [inputs["global_Wv"], inputs["rel_Wv"], inputs["local_Wv"]],
    }

    p = np.arange(128)[:, None]
    q = np.arange(128)[None, :]
    tri01 = (p <= q).astype(np.float32)       # j<=i within diag block
    win01 = (p >= q).astype(np.float32)       # j>=i-128 within prev block

    f8 = ml_dtypes.float8_e4m3

    def relv(h, d):
        return rel_emb[h, np.clip(d, -MRP, MRP) + MRP]

    in_maps = []
    for c in range(8):
        b, h = c // 2, c % 2
        rel0 = float(rel_emb[h, 0])
        # multiplicative post-exp masks: exp of the additive bias, with 0 at
        # causal/local cutoffs
        m = np.zeros((NT, 2, 128, 128), np.float32)  # cast to bf16 below
        m[0, 0] = tri01
        m[1, 0] = np.exp(relv(h, p - q) - rel0) * tri01
        m[1, 1] = np.exp(relv(h, p - q - 128) - rel0)
        m[2, 0] = tri01
        m[2, 1] = win01
        bv = np.zeros((128, NT), np.float32)
        bv[:, 1] = rel0

        wq_ = np.stack(
            [np.asarray(w_by_type["q"][t][h], np.float32) for t in range(NT)]
        )
        wk_ = np.stack(
            [np.asarray(w_by_type["k"][t][h], np.float32) for t in range(NT)]
        )
        wv_ = np.concatenate(
            [np.asarray(w_by_type["v"][t][h], np.float32) * 0.5 for t in range(NT)]
            + [router_W, np.zeros((D, 1), np.float32)],
            axis=1,
        )
        in_maps.append(
            {
                "xT": np.ascontiguousarray(x[b].T),
                "x8T": np.ascontiguousarray(x[b].T).astype(f8),
                "wq": np.ascontiguousarray(wq_ * WS).astype(f8),
                "wk": np.ascontiguousarray(wk_ * WS).astype(f8),
                "wv": np.ascontiguousarray(wv_),
                "masks": m.astype(ml_dtypes.bfloat16),
                "biasv": bv,
                "bb": np.broadcast_to(router_b, (128, NT)).copy(),
                "Wo": np.ascontiguousarray(W_o).astype(ml_dtypes.bfloat16),
                "wob": np.broadcast_to(W_o_b, (128, D)).copy(),
            }
        )
    return in_maps


def kernel(**inputs) -> np.ndarray:
    from concourse.bass_utils import run_bass_kernel_spmd

    if "nc" not in _CACHE:
        _CACHE["nc"] = _build_nc()
    nc = _CACHE["nc"]

    in_maps = _prep_inputs(inputs)
    trace = os.environ.get("KERNEL_TRACE", "0") == "1"
    res = run_bass_kernel_spmd(
        nc, in_maps, core_ids=list(range(8)), trace=trace
    )
    _CACHE["last_result"] = res

    out = np.empty((B, T, D), np.float32)
    for c in range(8):
        b, h = c // 2, c % 2
        r = res.results[c]["out"].reshape(NB // 2, 128, D)
        for j in range(NB // 2):
            blk = 2 * j + h
            out[b, 128 * blk : 128 * (blk + 1), :] = r[j]
    return out


# revision 129
# speedup vs baseline: 1.0118x; 1.0003x over previous
"""Trainium2 Bass kernel for nn_MixtureOfAttentionHeads.

Sharding: 8 cores = 4 batches x 2 heads. Core c handles batch c//2, head c%2:
it computes all three attention types (global/rel/local) for its head over the
full sequence, applies the router gate per token, then pairwise ReduceScatters
sum the two heads (0.5 factor folded into Wv); each core projects its owned
token blocks with W_o and writes them to DRAM.

Main optimizations over the straightforward version:
- Q/K projections in fp8e4m3 DoubleRow mode (4 accumulation steps over the
  256-deep double rows instead of 8, at 0.5 cycles/row). Weights are
  pre-scaled x32 on the host; the 32*32=1024 score scale folds into the
  exp scale, so fp8 costs no extra on-chip ops.
- V/router projection stays f32r: top-k routing decisions flip on ~1e-3
  logit perturbations and single flips produce O(10%) point errors, so the
  router path needs near-fp32 logits.
- Rel-position bias and causal/local masks are applied MULTIPLICATIVELY
  after the exp (es *= exp(bias) tiles precomputed on host) instead of as
  additive mask matmuls - frees the PE dispatch stream.
- Interleaved block ownership (rank h owns blocks with block%2==h) so every
  ReduceScatter piece carries tokens of both ranks; 3 pieces sized (4,8,4)
  blocks fire as their chunks finish, hiding all but the last collective's
  15us fixed cost.
- Per-chunk schedule staged as S->exp (only needs Q/K) ahead of the V
  projection so the tensor engine never idles on the f32r x DMA stream, and
  late chunks interleave next-type S phases with AV phases to keep the
  activation engine's exp stream saturated.

The SPMD program is identical on every core; all per-core differences
(batch, head weights, rel-bias tiles) arrive as input data.
"""

import os
import sys

sys.path.insert(0, "/opt/trn_rl_repo")

import numpy as np
import ml_dtypes

# --- problem constants (hardcoded per contract) ---
B, T, D, DH = 4, 2048, 1024, 128
H, NT, TOPK = 2, 3, 2
WIN, MRP = 128, 32
SCALE = float(np.sqrt(DH))
NB = T // 128          # 16 query/key blocks
NCH = T // 512         # 4 query chunks of 512
WS = 32.0              # fp8 weight prescale; scores come out x(WS*WS)
K2 = WS * WS           # 1024 score scale

_CACHE = {}

# RS pieces: (first block, nblocks). Rank h owns blocks with block%2==h, so
# every piece carries tokens for both ranks of the pair.
PIECES = [(0, 4), (4, 8), (12, 4)]


def _build_nc():
    import concourse.bass as bass  # noqa: F401
    import concourse.mybir as mybir
    import concourse.tile as tile
    from concourse import bacc
    from concourse.masks import make_identity
    from contextlib import ExitStack

    dt = mybir.dt
    f32, f32r, bf16, f8 = dt.float32, dt.float32r, dt.bfloat16, dt.float8e4
    AX = mybir.AxisListType
    ALU = mybir.AluOpType
    ACTF = mybir.ActivationFunctionType
    DR = mybir.MatmulPerfMode.DoubleRow

    nc = bacc.Bacc("TRN2", target_bir_lowering=False, num_devices=8)

    xT = nc.dram_tensor("xT", [D, T], f32r, kind="ExternalInput")
    x8T = nc.dram_tensor("x8T", [D, T], f8, kind="ExternalInput")
    wq = nc.dram_tensor("wq", [NT, D, DH], f8, kind="ExternalInput")
    wk = nc.dram_tensor("wk", [NT, D, DH], f8, kind="ExternalInput")
    wv = nc.dram_tensor("wv", [D, NT * DH + NT + 1], f32r, kind="ExternalInput")
    masks = nc.dram_tensor("masks", [NT, 2, 128, 128], bf16, kind="ExternalInput")
    biasv = nc.dram_tensor("biasv", [128, NT], f32, kind="ExternalInput")
    bb = nc.dram_tensor("bb", [128, NT], f32, kind="ExternalInput")
    Wo = nc.dram_tensor("Wo", [DH, D], bf16, kind="ExternalInput")
    wob = nc.dram_tensor("wob", [128, D], f32, kind="ExternalInput")
    out = nc.dram_tensor("out", [T // 2, D], f32, kind="ExternalOutput")
    dbg = os.environ.get("KERNEL_DEBUG", "0") == "1"
    if dbg:
        dbg_wgt = nc.dram_tensor("dbg_wgt", [128, NB, NT], f32, kind="ExternalOutput")

    NV = NT * DH + NT + 1  # 388 (even, fp32r requirement)

    with tile.TileContext(nc) as tc, ExitStack() as ctx:
        persist = ctx.enter_context(tc.tile_pool(name="persist", bufs=1))
        xsp = ctx.enter_context(tc.tile_pool(name="xsp", bufs=8))
        x8p = ctx.enter_context(tc.tile_pool(name="x8p", bufs=2))
        gatep = ctx.enter_context(tc.tile_pool(name="gatep", bufs=8))
        expp = ctx.enter_context(tc.tile_pool(name="expp", bufs=44))
        avs = ctx.enter_context(tc.tile_pool(name="avs", bufs=6))
        dram = ctx.enter_context(tc.tile_pool(name="dram", bufs=1, space="DRAM"))
        php = ctx.enter_context(tc.tile_pool(name="php", bufs=2, space="PSUM"))
        phv = ctx.enter_context(tc.tile_pool(name="phv", bufs=2, space="PSUM"))
        sp = ctx.enter_context(tc.tile_pool(name="sp", bufs=2, space="PSUM"))
        avp = ctx.enter_context(tc.tile_pool(name="avp", bufs=2, space="PSUM"))

        # ---- persistent SBUF tensors ----
        qT = [persist.tile([128, T], f32r, tag=f"qT{t}", name=f"qT{t}") for t in range(NT)]
        kT = [persist.tile([128, T], f32r, tag=f"kT{t}", name=f"kT{t}") for t in range(NT)]
        V3 = persist.tile([128, NB, NT, DH + 2], bf16, tag="V3")
        wgt = persist.tile([128, NB, NT], f32, tag="wgt")
        comb = persist.tile([128, NB, DH], f32, tag="comb")
        comb16 = persist.tile([128, NB, DH], bf16, tag="comb16")
        # [dh, parity, j, tok]: block 2j+hh lives at [:, hh, j, :]
        combT = persist.tile([128, 2, NB // 2, 128], bf16, tag="combT")
        mask_sb = persist.tile([128, NT, 2, 128], bf16, tag="masks")
        biasv_sb = persist.tile([128, NT], f32, tag="biasv")
        bb_sb = persist.tile([128, NT], f32, tag="bb")
        Wo_sb = persist.tile([128, D], bf16, tag="Wo")
        wob_sb = persist.tile([128, D], f32, tag="wob")
        ident16 = persist.tile([128, 128], bf16, tag="ident16")

        wq_sb = persist.tile([128, NT, 8, DH], f8, tag="wq")
        wk_sb = persist.tile([128, NT, 8, DH], f8, tag="wk")
        wv_sb = persist.tile([128, 8, NV], f32r, tag="wv")

        def load_consts():
            # wv (big, needed early) on the scalar HWDGE queue; the other
            # constants ride the otherwise-idle SWDGE (gpsimd/Pool) path so
            # they don't contend with the x stream on the DMA engines
            nc.scalar.dma_start(
                out=wv_sb, in_=wv.rearrange("(k p) m -> p k m", p=128)
            )
            nc.scalar.dma_start(
                out=mask_sb, in_=masks.rearrange("t k p m -> p t k m")
            )
            nc.scalar.dma_start(out=biasv_sb, in_=biasv[:, :])
            nc.scalar.dma_start(out=bb_sb, in_=bb[:, :])
            # Wo/wob are needed only by the output projection (~45us in);
            # ship them on the idle SWDGE path off the main DMA engines
            nc.gpsimd.dma_start(out=Wo_sb, in_=Wo[:, :])
            nc.gpsimd.dma_start(out=wob_sb, in_=wob[:, :])

        make_identity(nc, ident16)
        nc.vector.memset(V3[:, :, :, DH : DH + 2], 0.0)
        nc.vector.memset(V3[:, :, :, DH : DH + 1], 1.0)

        rs_in = [
            dram.tile([2 * 128, 64 * nb], bf16, name=f"rs_in{p}")
            for p, (_, nb) in enumerate(PIECES)
        ]
        rs_out = [
            dram.tile([128, 64 * nb], bf16, name=f"rs_out{p}")
            for p, (_, nb) in enumerate(PIECES)
        ]

        def load_qk_weights_all():
            nc.sync.dma_start(
                out=wq_sb,
                in_=wq.rearrange("t (k p) m -> p t k m", p=128),
            )
            nc.scalar.dma_start(
                out=wk_sb,
                in_=wk.rearrange("t (k p) m -> p t k m", p=128),
            )

        xs_tiles = {}

        x8_tiles = {}

        def load_slab(c4):
            """DMA the fp8 + f32r x tiles for token slab c4 (512 toks)."""
            sl = slice(512 * c4, 512 * (c4 + 1))
            x8t = x8p.tile([128, 8, 512], f8, tag="x8", name=f"x8_{c4}")
            nc.sync.dma_start(
                out=x8t,
                in_=x8T.rearrange("(k p) m -> p k m", p=128)[:, :, sl],
            )
            x8_tiles[c4] = x8t
            xs = []
            for kk in range(4):  # k-pairs: half the HWDGE queue entries
                xp2 = xsp.tile([128, 2, 512], f32r, tag="xs",
                               name=f"xs_{c4}_{kk}")
                nc.sync.dma_start(
                    out=xp2,
                    in_=xT.rearrange("(k p) m -> p k m", p=128)[
                        :, 2 * kk : 2 * kk + 2, sl
                    ],
                )
                xs.append(xp2[:, 0, :])
                xs.append(xp2[:, 1, :])
            xs_tiles[c4] = xs

        def qk_part(c4):
            """fp8 DoubleRow Q/K projections for slab c4.

            NOTE: every weight-DMA must be ISSUED before the matmuls that
            read it (slab 0 consumes all three types) — first-run reads of
            not-yet-written SBUF otherwise.
            """
            sl = slice(512 * c4, 512 * (c4 + 1))
            x8t = x8_tiles.pop(c4)
            for t in range(NT):
                psq = php.tile([128, 512], f32, tag="php")
                for k in range(4):
                    nc.tensor.matmul(
                        psq,
                        wq_sb[:, t, 2 * k : 2 * k + 2, :],
                        x8t[:, 2 * k : 2 * k + 2, :],
                        start=(k == 0), stop=(k == 3),
                        perf_mode=DR,
                    )
                nc.scalar.copy(qT[t][:, sl], psq)
                psk = php.tile([128, 512], f32, tag="php")
                for k in range(4):
                    nc.tensor.matmul(
                        psk,
                        wk_sb[:, t, 2 * k : 2 * k + 2, :],
                        x8t[:, 2 * k : 2 * k + 2, :],
                        start=(k == 0), stop=(k == 3),
                        perf_mode=DR,
                    )
                nc.vector.tensor_copy(kT[t][:, sl], psk)

        def v_part(c4):
            """V + router projection and gates for slab c4 (needs xs)."""
            xs = xs_tiles.pop(c4)
            lg4 = gatep.tile([128, 4, NT], f32, tag="lg4")
            for ib in range(4):
                i = 4 * c4 + ib
                ibo = 128 * ib
                psv = phv.tile([128, NV], f32, tag="phv")
                for k in range(8):
                    nc.tensor.matmul(
                        psv,
                        xs[k][:, ibo : ibo + 128],
                        wv_sb[:, k, :],
                        start=(k == 0),
                        stop=(k == 7),
                    )
                nc.vector.tensor_copy(V3[:, i, :, 0:DH], psv[:, 0 : NT * DH])
                nc.vector.tensor_add(
                    lg4[:, ib, :], psv[:, NT * DH : NT * DH + NT], bb_sb
                )
            # vectorized gate pipeline over the whole slab (4 blocks at once)
            mn = gatep.tile([128, 4, 1], f32, tag="mn")
            nc.vector.tensor_tensor(
                mn, lg4[:, :, 0:1], lg4[:, :, 1:2], op=ALU.min
            )
            mn2 = gatep.tile([128, 4, 1], f32, tag="mn2")
            nc.vector.tensor_tensor(mn2, mn, lg4[:, :, 2:3], op=ALU.min)
            eg = gatep.tile([128, 4, NT], f32, tag="eg")
            nc.scalar.activation(eg, lg4, ACTF.Exp)
            gt = gatep.tile([128, 4, NT], f32, tag="gt")
            nc.vector.tensor_tensor(
                gt, lg4, mn2.broadcast_to([128, 4, NT]), op=ALU.is_gt
            )
            ew = gatep.tile([128, 4, NT], f32, tag="ew")
            nc.vector.tensor_mul(ew, eg, gt)
            sm = gatep.tile([128, 4, 1], f32, tag="sm")
            nc.vector.tensor_reduce(sm, ew, axis=AX.X, op=ALU.add)
            rc = gatep.tile([128, 4, 1], f32, tag="rc")
            nc.vector.reciprocal(rc, sm)
            nc.vector.tensor_mul(
                wgt[:, 4 * c4 : 4 * c4 + 4, :], ew, rc.broadcast_to([128, 4, NT])
            )

        def s_phase(t, c4):
            """S^T -> exp -> masked es tiles for type t, query chunk c4."""
            is_causal = t < 2
            has_prev = t >= 1
            base = 4 * c4
            jlo_c = 0 if is_causal else max(0, base - 1)
            es_tiles = {}

            def s_matmul(ps, j, cb, off, hi):
                """S^T matmul into ps[:, cb:cb+512] (masks applied post-exp)."""
                # widen narrow f32r matmuls to >=256 cols (4x rate penalty
                # below 256); extra cols are never read downstream
                off_mm = off if hi - off >= 256 else max(0, hi - 256)
                nc.tensor.matmul(
                    ps[:, cb + off_mm : cb + hi],
                    kT[t][:, 128 * j : 128 * (j + 1)],
                    qT[t][:, 512 * c4 + off_mm : 512 * c4 + hi],
                    start=True,
                    stop=True,
                )

            def do_exp(ps, es, j, lo, hi):
                nc.scalar.activation(
                    es[:, lo:hi],
                    ps[:, lo:hi],
                    ACTF.Exp,
                    bias=biasv_sb[:, t : t + 1],
                    scale=1.0 / (K2 * SCALE),
                )
                # multiplicative masks (exp of additive bias, incl 0 for
                # causal/local cutoffs) over the diagonal-band windows
                has_diag = j >= base
                has_pr = has_prev and base <= j + 1 <= base + 3
                if has_diag and has_pr:
                    col = 128 * (j - base)
                    nc.vector.tensor_mul(
                        es[:, col : col + 256], es[:, col : col + 256],
                        mask_sb[:, t, :, :],
                    )
                elif has_diag:
                    col = 128 * (j - base)
                    nc.vector.tensor_mul(
                        es[:, col : col + 128], es[:, col : col + 128],
                        mask_sb[:, t, 0, :],
                    )
                elif has_pr:
                    col = 128 * (j + 1 - base)
                    nc.vector.tensor_mul(
                        es[:, col : col + 128], es[:, col : col + 128],
                        mask_sb[:, t, 1, :],
                    )

            for j in range(jlo_c, base + 4):
                off = 128 * max(0, j - base)
                hi = 512 if is_causal else 128 * min(4, (j - base) + 2)
                ps = sp.tile([128, 512], f32, tag="spsum")
                es = expp.tile([128, 512], bf16, tag="es")
                s_matmul(ps, j, 0, off, hi)
                do_exp(ps, es, j, off, hi)
                es_tiles[j] = (es, 0)
            return es_tiles

        def av_phase(t, c4, es_tiles):
            """AV + gated combine for type t, query chunk c4."""
            is_causal = t < 2
            base = 4 * c4
            for i in range(base, base + 4):
                jlo_i = 0 if is_causal else max(0, i - 1)
                pav = avp.tile([128, DH + 2], f32, tag="avpsum")
                col = 128 * (i - base)
                for j in range(jlo_i, i + 1):
                    est, cb = es_tiles[j]
                    nc.tensor.matmul(
                        pav,
                        est[:, cb + col : cb + col + 128],
                        V3[:, j, t, :],
                        start=(j == jlo_i),
                        stop=(j == i),
                    )
                rc = avs.tile([128, 1], f32, tag="rcav")
                nc.vector.reciprocal(rc, pav[:, DH : DH + 1])
                if t == 0:
                    nc.vector.tensor_scalar(
                        comb[:, i, :], pav[:, 0:DH], rc, wgt[:, i, t : t + 1],
                        op0=ALU.mult, op1=ALU.mult,
                    )
                else:
                    rcw = avs.tile([128, 1], f32, tag="rcw")
                    nc.vector.tensor_mul(rcw, rc, wgt[:, i, t : t + 1])
                    dst = comb if t == 1 else comb16
                    nc.vector.scalar_tensor_tensor(
                        dst[:, i, :], pav[:, 0:DH], rcw, comb[:, i, :],
                        op0=ALU.mult, op1=ALU.add,
                    )
                    if t == 2:
                        # block finished: transpose to [dh, tok] for the exchange
                        tp = avp.tile([128, 128], bf16, tag="avpsum")
                        nc.tensor.transpose(tp, comb16[:, i, :], ident16)
                        nc.vector.tensor_copy(combT[:, i % 2, i // 2, :], tp)

        def do_rs(p):
            """ReduceScatter piece p over the core pair. rs_in rows: first 128
            = rank0's (even) blocks, next 128 = rank1's (odd) blocks."""
            base, nb = PIECES[p]
            j0 = base // 2
            for jj in range(nb // 2):
                nc.scalar.dma_start(
                    out=rs_in[p].rearrange("(hh p) (j m) -> p hh j m", hh=2, m=128)[
                        :, :, jj, :
                    ],
                    in_=combT[:, :, j0 + jj, :],
                )
            nc.gpsimd.collective_compute(
                "ReduceScatter",
                mybir.AluOpType.add,
                replica_groups=[[0, 1], [2, 3], [4, 5], [6, 7]],
                ins=[rs_in[p].opt()],
                outs=[rs_out[p].opt()],
            )

        # ---------------- main schedule: slabs interleaved with attention ----
        load_qk_weights_all()
        load_slab(0)
        load_consts()
        load_slab(1)
        for c4 in range(NCH):
            qk_part(c4)
            if c4 < 2:
                # early chunks: run S/exp (qT/kT only) while the xs stream
                # is still landing, then V-proj, then AV
                es_all = [s_phase(t, c4) for t in range(NT)]
                v_part(c4)
                if c4 + 2 < NCH:
                    load_slab(c4 + 2)
                for t in range(NT):
                    av_phase(t, c4, es_all[t])
            else:
                v_part(c4)
                if c4 + 2 < NCH:
                    load_slab(c4 + 2)
                # keep the Act exp stream fed: next type's S phase is
                # emitted before the previous type's AV phase
                es0 = s_phase(0, c4)
                es1 = s_phase(1, c4)
                av_phase(0, c4, es0)
                es2 = s_phase(2, c4)
                av_phase(1, c4, es1)
                if c4 == 3:
                    with tc.high_priority(offset=400):
                        av_phase(2, c4, es2)
                else:
                    av_phase(2, c4, es2)
            if c4 == 0:
                do_rs(0)
            elif c4 == 2:
                do_rs(1)

        # ---------------- output projection per RS piece ---------------------
        with tc.tile_pool(name="fin", bufs=6) as fin:
            if dbg:
                nc.scalar.dma_start(out=dbg_wgt[:, :, :], in_=wgt[:, :, :])

            def fin_piece(p):
                pbase, nb = PIECES[p]
                halfT = fin.tile([128, 64 * nb], bf16, tag="halfT",
                                 name=f"halfT{p}")
                nc.sync.dma_start(out=halfT, in_=rs_out[p][:, :])
                for i in range(nb // 2):  # owned blocks in this piece
                    # owned block index within full sequence: pbase + 2*i + h
                    # (h-dependence handled on the host side when scattering)
                    row0 = (pbase // 2 + i) * 128
                    for n2 in range(2):
                        nsl = slice(512 * n2, 512 * (n2 + 1))
                        pf = php.tile([128, 512], f32, tag="php")
                        nc.tensor.matmul(
                            pf,
                            halfT[:, 128 * i : 128 * (i + 1)],
                            Wo_sb[:, nsl],
                            start=True,
                            stop=True,
                        )
                        ob = fin.tile([128, 512], f32, tag="ob")
                        nc.vector.tensor_add(ob, pf, wob_sb[:, nsl])
                        eng = nc.sync if n2 == 0 else nc.scalar
                        eng.dma_start(out=out[row0 : row0 + 128, nsl], in_=ob)

            # pieces 0/1 project while the last piece's collective runs;
            # do_rs(2) is emitted after them so its long semaphore wait
            # doesn't block their queue entries
            fin_piece(0)
            fin_piece(1)
            with tc.high_priority(offset=200):
                do_rs(2)
            fin_piece(2)

    nc.compile()
    return nc


def _prep_inputs(inputs):
    """Build the 8 per-core input maps from the full problem inputs."""
    x = np.asarray(inputs["x"], dtype=np.float32)
    rel_emb = np.asarray(inputs["rel_emb"], dtype=np.float32)
    router_W = np.asarray(inputs["router_W"], dtype=np.float32)
    router_b = np.asarray(inputs["router_b"], dtype=np.float32)
    W_o = np.asarray(inputs["W_o"], dtype=np.float32)
    W_o_b = np.asarray(inputs["W_o_b"], dtype=np.float32)

    # job order is (global, rel, local) = reference type indices (1, 2, 0);
    # permute router columns so logit column t matches job t
    perm = [1, 2, 0]
    router_W = router_W[:, perm]
    router_b = router_b[perm]

    w_by_type = {
        "q": [inputs["global_Wq"], inputs["rel_Wq"], inputs["local_Wq"]],
        "k": [inputs["global_Wk"], inputs["rel_Wk"], inputs["local_Wk"]],
        "v": [inputs["global_Wv"], inputs["rel_Wv"], inputs["local_Wv"]],
    }

    p = np.arange(128)[:, None]
    q = np.arange(128)[None, :]
    tri01 = (p <= q).astype(np.float32)       # j<=i within diag block
    win01 = (p >= q).astype(np.float32)       # j>=i-128 within prev block

    f8 = ml_dtypes.float8_e4m3

    def relv(h, d):
        return rel_emb[h, np.clip(d, -MRP, MRP) + MRP]

    in_maps = []
    for c in range(8):
        b, h = c // 2, c % 2
        rel0 = float(rel_emb[h, 0])
        # multiplicative post-exp masks: exp of the additive bias, with 0 at
        # causal/local cutoffs
        m = np.zeros((NT, 2, 128, 128), np.float32)  # cast to bf16 below
        m[0, 0] = tri01
        m[1, 0] = np.exp(relv(h, p - q) - rel0) * tri01
        m[1, 1] = np.exp(relv(h, p - q - 128) - rel0)
        m[2, 0] = tri01
        m[2, 1] = win01
        bv = np.zeros((128, NT), np.float32)
        bv[:, 1] = rel0

        wq_ = np.stack(
            [np.asarray(w_by_type["q"][t][h], np.float32) for t in range(NT)]
        )
        wk_ = np.stack(
            [np.asarray(w_by_type["k"][t][h], np.float32) for t in range(NT)]
        )
        wv_ = np.concatenate(
            [np.asarray(w_by_type["v"][t][h], np.float32) * 0.5 for t in range(NT)]
            + [router_W, np.zeros((D, 1), np.float32)],
            axis=1,
        )
        in_maps.append(
            {
                "xT": np.ascontiguousarray(x[b].T),
                "x8T": np.ascontiguousarray(x[b].T).astype(f8),
                "wq": np.ascontiguousarray(wq_ * WS).astype(f8),
                "wk": np.ascontiguousarray(wk_ * WS).astype(f8),
                "wv": np.ascontiguousarray(wv_),
                "masks": m.astype(ml_dtypes.bfloat16),
                "biasv": bv,
                "bb": np.broadcast_to(router_b, (128, NT)).copy(),
                "Wo": np.ascontiguousarray(W_o).astype(ml_dtypes.bfloat16),
                "wob": np.broadcast_to(W_o_b, (128, D)).copy(),
            }
        )
    return in_maps


def kernel(**inputs) -> np.ndarray:
    from concourse.bass_utils import run_bass_kernel_spmd

    if "nc" not in _CACHE:
        _CACHE["nc"] = _build_nc()
    nc = _CACHE["nc"]

    in_maps = _prep_inputs(inputs)
    trace = os.environ.get("KERNEL_TRACE", "0") == "1"
    res = run_bass_kernel_spmd(
        nc, in_maps, core_ids=list(range(8)), trace=trace
    )
    _CACHE["last_result"] = res

    out = np.empty((B, T, D), np.float32)
    for c in range(8):
        b, h = c // 2, c % 2
        r = res.results[c]["out"].reshape(NB // 2, 128, D)
        for j in range(NB // 2):
            blk = 2 * j + h
            out[b, 128 * blk : 128 * (blk + 1), :] = r[j]
    return out
